# revision 4
# baseline (speedup 1.0000x reference)
"""GPTQ int4 linear (nn_GPTQLinear) on 8 TRN2 NeuronCores — Bass/Tile kernel.

Full problem: x [4, 2048, 4096] fp32, packed int4 weights [4096 x 4096],
groupwise dequant (group size 128), y = x @ W.T + bias -> [4, 2048, 4096].

Sharding: 2-way data-parallel on x rows x 4-way tensor-parallel on
out_features.  Per core: M=4096 rows, O=1024 out features, K=4096, all
matmuls fp16 FD=512 (2048 MMs/core ~= 437us MAC floor).

Per-core rep structure (vs the previous chunk-paced-groups kernel this
measured -32 +- 5 us/rep on hardware, A/B interleaved wall-clock):
  [chunk-paced g0 sweep, mts 0..G0-1]  [pair-interleaved mt-major middle]
  [chunk-paced final group, mts NM-GF..NM-1]
 - Weights live as 4 chunks (out-tile t x k-half q) of [128, 16, 512] fp16;
   chunk (0,0) double-buffered.  The final group frees chunks gradually so
   the next rep's dequant + transposes overlap the boundary; the g0 sweep
   consumes them slowly while the remaining (1,*) dequant lands.
 - The mt-major middle processes m-tiles in PAIRS, alternating the two
   PSUM banks MM-to-MM: a long run of back-to-back accumulating matmuls
   into a single PSUM bank measured ~35us/rep slower on HW (read-modify-
   write turnaround); alternating banks hides it.  Pairs also keep xt
   lifetimes ~55us so the 9-deep xt ring never starves.
 - Chunk-paced groups evict via ACT copies from bias-prefilled PSUM (K=1
   ones x bias16 matmuls) so the DVE queue at rep boundaries carries only
   dequant; the mt-major middle evicts via DVE tensor_tensor(+bias).
 - Packed-weight/zero/scale loads are 3D-AP HWDGE DMAs on the scalar
   queue; x fp32->fp16 cast-loads on gpsimd SWDGE; all transposes
   (weights + x) on the sync-queue xbar, ordered by a single global
   scheduler-only chain.
"""

import sys

if "/opt/trn_rl_repo" not in sys.path:
    sys.path.insert(0, "/opt/trn_rl_repo")

import numpy as np
from contextlib import ExitStack

import concourse.bass as bass
import concourse.tile as tile
import concourse.mybir as mybir
from bass_rust import ScopedClock

F32 = mybir.dt.float32
F16 = mybir.dt.float16
U8 = mybir.dt.uint8
AOP = mybir.AluOpType

B, S, IN_F, OUT_F = 4, 2048, 4096, 4096
GS = 128
N_CORES = 8
DP, TP = 2, 4
M_CORE = (B * S) // DP         # 4096
O_CORE = OUT_F // TP           # 1024
K = IN_F


def _split_multi_waits(nc, max_waits=1):
    # walrus in this container rejects >1 sync-wait per instruction; move
    # extras onto same-engine InstNoOp carriers (in-order execution keeps
    # semantics).
    n_split = 0
    for fn in nc.m.functions:
        for blk in fn.blocks:
            insts = list(blk.instructions)
            out = []
            for inst in insts:
                si = inst.sync_info
                if (si is not None and len(si.on_wait) > max_waits
                        and inst.engine is not None):
                    w = list(si.on_wait)
                    keep = w[-max_waits:]
                    for j, wx in enumerate(w[:-max_waits]):
                        nop = mybir.InstNoOp(name=f"{inst.name}-w{j}",
                                             ins=[], outs=[])
                        nop.engine = inst.engine
                        nop.sync_info = mybir.SyncInfo(on_wait=[wx],
                                                       on_update=[])
                        nc.register_instruction(nop, overwrite=True)
                        out.append(nop)
                    si.on_wait = keep
                    n_split += 1
                out.append(inst)
            blk.instructions = out
    return n_split


def _patched_drain_and_barrier(self, tick_clock, wait_clock):
    # split the final drain's multi-waits across chained drains (same walrus
    # limitation as above).
    nc = self.nc
    drain_inst = nc.sync.drain()
    wait_clock.add_sem_waits(drain_inst.ins,
                             ScopedClock({None: tick_clock.global_clock}))
    si = drain_inst.ins.sync_info
    if si is not None:
        w = list(si.on_wait)
        if len(w) > 1:
            si.on_wait = w[:1]
            for extra in w[1:]:
                d2 = nc.sync.drain()
                d2.ins.sync_info = mybir.SyncInfo(on_wait=[extra], on_update=[])
    nc.all_engine_barrier()
    assert self.sems is not None
    popped = nc._tile_sem_poison_stack.pop()
    assert popped is self._sem_poison
    nc.clear_and_free_semaphores(list(self.sems.allocated().values()))
    nc.all_engine_barrier()


tile.TileContext._drain_and_barrier = _patched_drain_and_barrier


def build_nc(M=M_CORE, K_=K, O=O_CORE, reps=1, NQ=2, xtb=9, x16b=2,
             wblkb=3, tpkb=2, yb=3, G0=4, GF=4, xahead=3, G0F=8,
             mtg=2, dbl=True, swi=False):
    P = 128
    NK = K_ // P            # 32 k-tiles == quant groups
    NM = M // P             # 32 m-tiles
    OT = 512                # out-tile width (one fp32 PSUM bank)
    NOT = O // OT           # 2 out-tiles
    OBT = OT // P           # 4 o-blocks per out-tile
    NOB = O // P            # 8 o-blocks
    KB = K_ // 2            # 2048 packed bytes per o-row
    NKC = NK // NQ          # k-tiles per chunk
    BPC = KB // NQ          # packed bytes per chunk per row
    BPG = GS // 2           # 64 packed bytes per group
    CHUNKS = [(t, q) for t in range(NOT) for q in range(NQ)]

    nc = bass.Bass("TRN2", target_bir_lowering=False, debug=False,
                   enable_asserts=False)

    xs = nc.dram_tensor("xs", [M, K_], F32, kind="ExternalInput")
    pk = nc.dram_tensor("pk", [O, KB], U8, kind="ExternalInput")
    sc = nc.dram_tensor("sc", [O, NK], F32, kind="ExternalInput")
    zr = nc.dram_tensor("zr", [O, NK], F32, kind="ExternalInput")
    bs = nc.dram_tensor("bs", [O], F32, kind="ExternalInput")
    yo = nc.dram_tensor("yo", [M, O], F32, kind="ExternalOutput")

    with tile.TileContext(nc) as tc, ExitStack() as ctx:
        wt_pool = ctx.enter_context(tc.tile_pool(name="wt", bufs=1))
        wt2_pool = ctx.enter_context(tc.tile_pool(name="wt2", bufs=2))
        wst_pool = ctx.enter_context(tc.tile_pool(name="wst", bufs=wblkb))
        tpk_pool = ctx.enter_context(tc.tile_pool(name="tpkp", bufs=tpkb))
        tmp_pool = ctx.enter_context(tc.tile_pool(name="tmp", bufs=2))
        sz_pool = ctx.enter_context(tc.tile_pool(name="sz", bufs=1))
        x_pool = ctx.enter_context(tc.tile_pool(name="x", bufs=x16b))
        xt_pool = ctx.enter_context(tc.tile_pool(name="xt", bufs=xtb))
        y_pool = ctx.enter_context(tc.tile_pool(name="y", bufs=yb))
        c_pool = ctx.enter_context(tc.tile_pool(name="c", bufs=1))
        ps_pool = ctx.enter_context(tc.tile_pool(name="ps", bufs=8, space="PSUM"))

        ones = c_pool.tile([1, P], F16, tag="ones")
        nc.vector.memset(ones[:], 1.0)
        bias16 = c_pool.tile([1, O], F16, tag="bias16")
        nc.gpsimd.dma_start(bias16[:], bs[None, :])  # cast f32->f16
        bias_bc = c_pool.tile([P, O], F16, tag="bias_bc")
        for t in range(NOT):
            bp = ps_pool.tile([P, OT], F32, tag="ps", name=f"biasps{t}")
            nc.tensor.matmul(bp[:], ones[:], bias16[:, t * OT:(t + 1) * OT],
                             start=True, stop=True)
            nc.scalar.copy(bias_bc[:, t * OT:(t + 1) * OT], bp[:])

        # zero/scale: one 3D HWDGE DMA each, resident
        tz_all = sz_pool.tile([P, NOB, NK], F32, tag="tz")
        nc.scalar.dma_start(tz_all[:], zr.rearrange("(ob p) g -> p ob g", p=P))
        ts_all = sz_pool.tile([P, NOB, NK], F32, tag="ts")
        nc.scalar.dma_start(ts_all[:], sc.rearrange("(ob p) g -> p ob g", p=P))

        # one global transpose chain: scheduler-only edges keep the sync
        # queue dispatching in exactly this order (xbar race control)
        from concourse.tile import add_dep_helper
        chain = [None]

        def chained(binst):
            if chain[0] is not None:
                add_dep_helper(binst.ins, chain[0].ins, sync=False,
                               reason="sync-queue transpose order")
            chain[0] = binst
            return binst

        def new_state(rep):
            rn = f"r{rep}"
            wts = {(t, q): (wt2_pool if dbl and t == 0 and q < NQ // 2
                            else wt_pool).tile(
                                [P, NKC, OT], F16, tag=f"wt{t}{q}",
                                name=f"wt{t}{q}{rn}")
                   for (t, q) in CHUNKS}
            return {"rn": rn, "wts": wts, "xts": {}, "x16s": {}, "tpks": {},
                    "first": rep == 0, "xnext": 0}

        def load_tpk(st, t, q):
            tpk = tpk_pool.tile([P, OBT, BPC], U8, tag="tpk",
                                name=f"tpk{t}{q}{st['rn']}")
            src = pk[t * OT:(t + 1) * OT, q * BPC:(q + 1) * BPC]
            nc.scalar.dma_start(tpk[:], src.rearrange("(ob p) b -> p ob b", p=P))
            st["tpks"][(t, q)] = tpk

        def load_x16(st, mt):
            x16 = x_pool.tile([P, K_], F16, tag="x16",
                              name=f"x16m{mt}{st['rn']}")
            nc.gpsimd.dma_start(x16[:], xs[mt * P:(mt + 1) * P, :])
            st["x16s"][mt] = x16

        def make_xt(st, mt):
            xt = xt_pool.tile([P, NK, P], F16, tag="xt",
                              name=f"xt{mt}{st['rn']}")
            chained(nc.sync.dma_start_transpose(xt[:], st["x16s"][mt][:]))
            st["xts"][mt] = xt

        def ensure_x(st, upto):
            while st["xnext"] <= min(upto, NM - 1):
                mt = st["xnext"]
                if mt not in st["x16s"]:
                    load_x16(st, mt)
                if mt not in st["xts"]:
                    make_xt(st, mt)
                st["xnext"] = mt + 1

        def dequant_unit(st, t, q, ob):
            # [128o, BPC packed bytes] -> wblk [128o, 2*BPC] f16 (unpack
            # nibbles, subtract zero, scale; stride-2 writes interleave the
            # lo/hi nibble streams), then transpose into the resident chunk
            tpk = st["tpks"][(t, q)]
            src = tpk[:, ob, :]
            zsl = tz_all[:, t * OBT + ob, q * NKC:(q + 1) * NKC]
            ssl = ts_all[:, t * OBT + ob, q * NKC:(q + 1) * NKC]
            zb = bass.AP(zsl.tensor, zsl.offset,
                         [zsl.ap[0], [1, NKC], [0, BPG]])
            sb = bass.AP(ssl.tensor, ssl.offset,
                         [ssl.ap[0], [1, NKC], [0, BPG]])

            lo_u8 = tmp_pool.tile([P, BPC], U8, tag="nib")
            nc.vector.tensor_scalar(lo_u8[:], src, 15, None,
                                    op0=AOP.bitwise_and)
            hi_u8 = tmp_pool.tile([P, BPC], U8, tag="nib")
            nc.vector.tensor_scalar(hi_u8[:], src, 4, None,
                                    op0=AOP.logical_shift_right)

            wblk = wst_pool.tile([P, 2 * BPC], F16, tag="wblk")
            wap = wblk[:]
            wev = bass.AP(wap.tensor, wap.offset,
                          [wap.ap[0], [GS, NKC], [2, BPG]])
            wod = bass.AP(wap.tensor, wap.offset + 1,
                          [wap.ap[0], [GS, NKC], [2, BPG]])

            tmp_lo = tmp_pool.tile([P, BPC], F16, tag="tmp")
            tlo = tmp_lo[:].rearrange("p (g b) -> p g b", g=NKC)
            nc.vector.scalar_tensor_tensor(
                tlo, lo_u8[:].rearrange("p (g b) -> p g b", g=NKC), 1.0,
                zb, op0=AOP.mult, op1=AOP.subtract)
            nc.vector.tensor_tensor(wev, tlo, sb, op=AOP.mult)

            tmp_hi = tmp_pool.tile([P, BPC], F16, tag="tmp")
            thi = tmp_hi[:].rearrange("p (g b) -> p g b", g=NKC)
            nc.vector.scalar_tensor_tensor(
                thi, hi_u8[:].rearrange("p (g b) -> p g b", g=NKC), 1.0,
                zb, op0=AOP.mult, op1=AOP.subtract)
            nc.vector.tensor_tensor(wod, thi, sb, op=AOP.mult)

            chained(nc.sync.dma_start_transpose(
                st["wts"][(t, q)][:, :, ob * P:(ob + 1) * P], wblk[:]))

        def dequant_chunk(st, t, q):
            for ob in range(OBT):
                dequant_unit(st, t, q, ob)

        def evict_act(st, mt, t, ps):
            o0 = t * OT
            yst = y_pool.tile([P, OT], F32, tag="yst")
            nc.scalar.copy(yst[:], ps[:])
            nc.scalar.dma_start(yo[mt * P:mt * P + P, o0:o0 + OT], yst[:])

        def evict_dve(st, mt, t, ps):
            o0 = t * OT
            yst = y_pool.tile([P, OT], F32, tag="yst")
            nc.vector.tensor_tensor(yst[:], ps[:], bias_bc[:, o0:o0 + OT],
                                    op=AOP.add)
            nc.scalar.dma_start(yo[mt * P:mt * P + P, o0:o0 + OT], yst[:])

        def mm(st, psum, mt, t, k, start, stop):
            q, kk = divmod(k, NKC)
            nc.tensor.matmul(psum[:], st["xts"][mt][:, k, :],
                             st["wts"][(t, q)][:, kk, :],
                             start=start, stop=stop)

        def chunk_sweep(st, mts, tag):
            """Chunk-paced group with prefilled bias + ACT eviction (keeps
            the DVE queue free for dequant around this group)."""
            rn = st["rn"]
            pss = {}
            for mt in mts:
                pss[(mt, 0)] = ps_pool.tile([P, OT], F32, tag="ps",
                                            name=f"ps{rn}{tag}m{mt}o0")
                nc.tensor.matmul(pss[(mt, 0)][:], ones[:],
                                 bias16[:, 0:OT], start=True, stop=False)
            for (t, q) in CHUNKS:
                if (t, q) == (1, 0):
                    for mt in mts:
                        pss[(mt, 1)] = ps_pool.tile(
                            [P, OT], F32, tag="ps",
                            name=f"ps{rn}{tag}m{mt}o1")
                        nc.tensor.matmul(pss[(mt, 1)][:], ones[:],
                                         bias16[:, OT:2 * OT],
                                         start=True, stop=False)
                if swi:
                    # interleave m-tiles so consecutive MMs alternate PSUM
                    # banks (same-bank accumulate turnaround penalty)
                    for kk in range(NKC):
                        k = q * NKC + kk
                        for mt in mts:
                            mm(st, pss[(mt, t)], mt, t, k,
                               start=False, stop=(k == NK - 1))
                    if q == NQ - 1:
                        for mt in mts:
                            evict_act(st, mt, t, pss[(mt, t)])
                else:
                    for mt in mts:
                        for kk in range(NKC):
                            k = q * NKC + kk
                            mm(st, pss[(mt, t)], mt, t, k,
                               start=False, stop=(k == NK - 1))
                        if q == NQ - 1:
                            evict_act(st, mt, t, pss[(mt, t)])

        def mt_group(st, mts):
            # pair-interleaved: consecutive MMs alternate PSUM banks, which
            # avoids the same-bank accumulate turnaround penalty
            rn = st["rn"]
            for t in range(NOT):
                pss = {mt: ps_pool.tile([P, OT], F32, tag="ps",
                                        name=f"ps{rn}m{mt}o{t}")
                       for mt in mts}
                for k in range(NK):
                    for mt in mts:
                        mm(st, pss[mt], mt, t, k, start=(k == 0),
                           stop=(k == NK - 1))
                for mt in mts:
                    evict_dve(st, mt, t, pss[mt])

        def emit_rep(st, nxt):
            mid_end = NM - GF
            if st["first"]:
                # startup: stagger pk loads, x loads, dequant and
                # transposes; the g0 sweep consumes chunks as they land
                for i, (tt, qq) in enumerate(CHUNKS):
                    load_tpk(st, tt, qq)
                    if i < 2:
                        load_x16(st, i)
                for i, (tt, qq) in enumerate(CHUNKS):
                    dequant_chunk(st, tt, qq)
                    if i + 2 < 7:
                        load_x16(st, i + 2)
                    make_xt(st, i)
                for i in range(len(CHUNKS), 7):
                    if i not in st["x16s"]:
                        load_x16(st, i)
                    make_xt(st, i)
                st["xnext"] = 7
                g0n = G0F
            else:
                g0n = G0
            ensure_x(st, g0n - 1)
            chunk_sweep(st, list(range(g0n)), tag="g0")
            mstart = g0n

            mids = list(range(mstart, mid_end))
            groups = [mids[i:i + mtg] for i in range(0, len(mids), mtg)]
            for gr in groups:
                ensure_x(st, gr[-1] + xahead)
                mt_group(st, gr)
                if nxt is not None and mid_end - 9 in gr:
                    for (tt, qq) in CHUNKS:
                        load_tpk(nxt, tt, qq)

            ensure_x(st, NM - 1)
            if nxt is not None:
                for q in range(NQ // 2):
                    dequant_chunk(nxt, 0, q)
                for m2 in range(G0):
                    load_x16(nxt, m2)
                    make_xt(nxt, m2)
                for q in range(NQ // 2, NQ):
                    dequant_chunk(nxt, 0, q)

            chunk_sweep(st, list(range(NM - GF, NM)), tag="gf")

            if nxt is not None:
                for q in range(NQ):
                    dequant_chunk(nxt, 1, q)
                nxt["xnext"] = G0

        states = [new_state(rep) for rep in range(reps)]
        for rep in range(reps):
            nxt = states[rep + 1] if rep + 1 < reps else None
            emit_rep(states[rep], nxt)

    _split_multi_waits(nc)
    return nc


_CACHED_NC = None


def _get_nc():
    global _CACHED_NC
    if _CACHED_NC is None:
        _CACHED_NC = build_nc()
    return _CACHED_NC


def make_in_maps(x, scale, zero, bias, packed_weight):
    x2 = np.ascontiguousarray(np.asarray(x, dtype=np.float32).reshape(B * S, IN_F))
    pk_all = np.asarray(packed_weight, dtype=np.int32)
    pk8 = np.ascontiguousarray(pk_all.view(np.uint8).reshape(OUT_F, IN_F // 2))
    scale = np.asarray(scale, dtype=np.float32)
    zero = np.asarray(zero, dtype=np.float32)
    bias = np.asarray(bias, dtype=np.float32)

    in_maps = []
    for c in range(N_CORES):
        mb, ob = c // TP, c % TP
        in_maps.append({
            "xs": np.ascontiguousarray(x2[mb * M_CORE:(mb + 1) * M_CORE]),
            "pk": np.ascontiguousarray(pk8[ob * O_CORE:(ob + 1) * O_CORE]),
            "sc": np.ascontiguousarray(scale[ob * O_CORE:(ob + 1) * O_CORE]),
            "zr": np.ascontiguousarray(zero[ob * O_CORE:(ob + 1) * O_CORE]),
            "bs": np.ascontiguousarray(bias[ob * O_CORE:(ob + 1) * O_CORE]),
        })
    return in_maps


def assemble(results):
    y = np.empty((B * S, OUT_F), dtype=np.float32)
    for c in range(N_CORES):
        mb, ob = c // TP, c % TP
        y[mb * M_CORE:(mb + 1) * M_CORE,
          ob * O_CORE:(ob + 1) * O_CORE] = results[c]["yo"]
    return y.reshape(B, S, OUT_F)


def kernel(x, scale, zero, bias, packed_weight, trace=False):
    from concourse.bass_utils import run_bass_kernel_spmd
    nc = _get_nc()
    in_maps = make_in_maps(x, scale, zero, bias, packed_weight)
    res = run_bass_kernel_spmd(nc, in_maps, core_ids=list(range(N_CORES)),
                               trace=trace)
    out = assemble(res.results)
    if trace:
        kernel.last_result = res
    return out
